# revision 15
# baseline (speedup 1.0000x reference)
"""Trainium2 Bass kernel for nn_CCModel (cross-correlation peak picker).

Math (per (b, c) row):
  X = conj(c1) * c2                       (complex cross-spectrum, F=2049 bins)
  xt = irfft(X, n=4096); roll; slice      -> 599 lags  [-299, 299]
  moving average over channels (window 20, zero-padded)
  peak pick (max vs |min|) -> cc, tshift

Device realization per pair:
  irfft+slice == matmul with cos/sin basis. Using lag symmetry:
    E[c,tau] = sum_k Xr[c,k] * (ck/N) cos(2 pi k tau / N)   tau in [0,300)
    O[c,tau] = sum_k Xi[c,k] * -(ck/N) sin(2 pi k tau / N)
    xcor[299+tau] = E + O ; xcor[299-tau] = E - O
  Xr = r1 r2 + i1 i2  (the add is fused into the PE transpose via PSUM
  accumulation), Xi = r1 i2 - i1 r2.
  Smoothing = banded [128,128] matmul W^T applied after combine.

All matmuls run in plain fp32: float32r (the full-rate PE mode) is only an
11-bit-mantissa format on TRN2, which would perturb the peak picks; fp32 at
4 cycles/row keeps tshift bit-exact vs the jax reference. Emission is
software-pipelined: each pair's combine/smooth/peak tail is deferred one
pair so the per-engine FIFOs never stall the next pair's products.

Sharding: pure data-parallel, 4 of the 32 batch pairs per NeuronCore.
"""

import sys

sys.path.insert(0, "/opt/trn_rl_repo")

import numpy as np
from contextlib import ExitStack

import concourse.bacc as bacc
import concourse.tile as tile
import concourse.mybir as mybir
from concourse.bass_utils import run_bass_kernel_spmd

dt = mybir.dt
Alu = mybir.AluOpType

# hardcoded problem shape
B, C, F = 32, 128, 2049
NCORES = 8
PB = B // NCORES  # pairs per core
NFAST = 4096
NLAG = 300  # int(3.0 / 0.01)
T = 2 * NLAG - 1  # 599
NMA = 20
DT_S = 0.01
TAU = NLAG  # 300 basis columns

# measured f32r rounding emulation for host-side constants (set from probe:
# number of mantissa bits kept; None -> keep full fp32 bits)
F32R_MBITS = None
F32R_RTN = True


def _round_f32r_host(a):
    if F32R_MBITS is None or F32R_MBITS >= 23:
        return a.astype(np.float32)
    bits = np.ascontiguousarray(a, dtype=np.float32).view(np.uint32)
    shift = np.uint32(23 - F32R_MBITS)
    mask = np.uint32(0xFFFFFFFF) << shift
    if F32R_RTN:
        half = np.uint32(1) << np.uint32(22 - F32R_MBITS)
        bits = (bits + half) & mask
    else:
        bits = bits & mask
    return bits.view(np.float32).copy()


def _build_consts():
    k = np.arange(F, dtype=np.int64)
    tau = np.arange(TAU, dtype=np.int64)
    ck = np.where((k == 0) | (k == NFAST // 2), 1.0, 2.0)
    phase = (k[:, None] * tau[None, :]) % NFAST  # exact ints
    ang = (2.0 * np.pi / NFAST) * phase.astype(np.float64)
    basisA = _round_f32r_host((ck[:, None] / NFAST) * np.cos(ang))
    basisB = _round_f32r_host(-(ck[:, None] / NFAST) * np.sin(ang))

    W = np.zeros((C, C), dtype=np.float64)
    pad = NMA // 2
    for c in range(C):
        W[c, max(0, c - pad) : min(C, c + pad)] = 1.0 / NMA
    WT = np.ascontiguousarray(W.T).astype(np.float32)
    ident = np.eye(128, dtype=np.float32)
    return basisA, basisB, WT, ident


def _build_bass():
    nc = bacc.Bacc("TRN2", target_bir_lowering=False, debug=False)
    d1 = nc.dram_tensor("data1", [PB, C, F, 2], dt.float32, kind="ExternalInput").ap()
    d2 = nc.dram_tensor("data2", [PB, C, F, 2], dt.float32, kind="ExternalInput").ap()
    bA = nc.dram_tensor("basisA", [F, TAU], dt.float32, kind="ExternalInput").ap()
    bB = nc.dram_tensor("basisB", [F, TAU], dt.float32, kind="ExternalInput").ap()
    wt = nc.dram_tensor("wt", [C, C], dt.float32, kind="ExternalInput").ap()
    idn = nc.dram_tensor("ident", [128, 128], dt.float32, kind="ExternalInput").ap()
    cc_o = nc.dram_tensor("cc", [PB, C], dt.float32, kind="ExternalOutput").ap()
    ts_o = nc.dram_tensor("tshift", [PB, C], dt.float32, kind="ExternalOutput").ap()

    with tile.TileContext(nc) as tc, ExitStack() as ctx:
        consts = ctx.enter_context(tc.tile_pool(name="consts", bufs=1))
        praw = ctx.enter_context(tc.tile_pool(name="raw", bufs=4))
        pprod = ctx.enter_context(tc.tile_pool(name="prod", bufs=2))
        pzt = ctx.enter_context(tc.tile_pool(name="zt", bufs=1))
        psmall = ctx.enter_context(tc.tile_pool(name="small", bufs=2))
        pout = ctx.enter_context(tc.tile_pool(name="outs", bufs=1))
        tp = ctx.enter_context(tc.tile_pool(name="tp", bufs=2, space="PSUM"))
        eo = ctx.enter_context(tc.tile_pool(name="eo", bufs=4, space="PSUM"))
        wp = ctx.enter_context(tc.tile_pool(name="wp", bufs=1, space="PSUM"))

        # resident constants (basis DMAs are emitted after pair 0's data
        # loads -- they are not needed until the first E matmul)
        baseA = consts.tile([128, 17 * TAU], dt.float32, tag="baseA")
        baseB = consts.tile([128, 17 * TAU], dt.float32, tag="baseB")

        def load_basis(dram, tile_):
            nc.sync.dma_start(
                tile_[:, 0 : 16 * TAU].rearrange("p (j t) -> p j t", j=16),
                dram[0 : 16 * 128].rearrange("(j p) t -> p j t", p=128),
            )
            nc.sync.dma_start(tile_[0:1, 16 * TAU : 17 * TAU], dram[2048:2049])

        wtt = consts.tile([128, 128], dt.float32, tag="wt")
        nc.sync.dma_start(wtt[:], wt)
        ident = consts.tile([128, 128], dt.float32, tag="ident")
        nc.sync.dma_start(ident[:], idn)

        ccs = pout.tile([128, PB], dt.float32, tag="ccs")
        tss = pout.tile([128, PB], dt.float32, tag="tss")

        def emit_head(b, pre_eo_hook=None):
            """DMA + products + transposes + E/O matmuls for pair b."""
            raw1 = praw.tile([128, F, 2], dt.float32, tag="raw")
            nc.sync.dma_start(raw1[:], d1[b])
            raw2 = praw.tile([128, F, 2], dt.float32, tag="raw")
            nc.sync.dma_start(raw2[:], d2[b])
            r1 = raw1[:, :, 0]
            i1 = raw1[:, :, 1]
            r2 = raw2[:, :, 0]
            i2 = raw2[:, :, 1]

            # cross-spectrum products; the t1+t2 add happens on the PE below
            t1 = pprod.tile([128, F], dt.float32, tag="t1")
            nc.vector.tensor_tensor(t1[:], r1, r2, Alu.mult)  # r1*r2
            t2 = pprod.tile([128, F], dt.float32, tag="t2")
            nc.gpsimd.tensor_tensor(t2[:], i1, i2, Alu.mult)  # i1*i2
            t3 = pprod.tile([128, F], dt.float32, tag="t3")
            nc.vector.tensor_tensor(t3[:], r1, i2, Alu.mult)  # r1*i2
            t4 = pprod.tile([128, F], dt.float32, tag="t4")
            nc.gpsimd.tensor_tensor(t4[:], i1, r2, Alu.mult)  # i1*r2
            zi = pprod.tile([128, F], dt.float32, tag="t4")
            nc.vector.tensor_tensor(zi[:], t3[:], t4[:], Alu.subtract)  # Xi

            def transposes(srcs, dstt):
                # transpose to k-major, accumulating multiple sources in PSUM
                for g in range(4):
                    pz = tp.tile([128, 512], dt.float32, tag="tp")
                    for q in range(4):
                        j = 4 * g + q
                        for si, src in enumerate(srcs):
                            nc.tensor.matmul(
                                pz[:, 128 * q : 128 * (q + 1)],
                                src[:, 128 * j : 128 * (j + 1)],
                                ident[:],
                                is_transpose=True,
                                start=(si == 0),
                                stop=(si == len(srcs) - 1),
                            )
                    nc.scalar.copy(dstt[:, 512 * g : 512 * (g + 1)], pz[:])
                # the k=2048 leftover row
                pz = tp.tile([128, 512], dt.float32, tag="tp")
                for si, src in enumerate(srcs):
                    nc.tensor.matmul(
                        pz[0:1, 0:128],
                        src[:, 2048:2049],
                        ident[:],
                        is_transpose=True,
                        start=(si == 0),
                        stop=(si == len(srcs) - 1),
                    )
                nc.scalar.copy(dstt[0:1, 16 * 128 : 17 * 128], pz[0:1, 0:128])

            def eo_matmuls(dstp, zt_, base):
                for j in range(16):
                    nc.tensor.matmul(
                        dstp[:],
                        zt_[:, 128 * j : 128 * (j + 1)],
                        base[:, TAU * j : TAU * (j + 1)],
                        start=(j == 0),
                        stop=False,
                    )
                nc.tensor.matmul(
                    dstp[:],
                    zt_[0:1, 16 * 128 : 17 * 128],
                    base[0:1, 16 * TAU : 17 * TAU],
                    start=False,
                    stop=True,
                )

            zrt = pzt.tile([128, 17 * 128], dt.float32, tag="zrt")
            transposes((t1, t2), zrt)
            if pre_eo_hook is not None:
                # pair 0: basis DMAs must be emitted before the first basis
                # read so Tile records the write->read dependency
                pre_eo_hook()
            eps = eo.tile([128, TAU], dt.float32, tag="eo")
            eo_matmuls(eps, zrt, baseA)
            zit = pzt.tile([128, 17 * 128], dt.float32, tag="zit")
            transposes((zi,), zit)
            ops_ = eo.tile([128, TAU], dt.float32, tag="eo")
            eo_matmuls(ops_, zit, baseB)
            return eps, ops_

        def emit_tail(b, eps, ops_):
            """combine + smoothing + peak pick for pair b (deferred a pair)."""
            # combine: xcor[299+tau] = E + O, xcor[299-tau] = E - O
            # (DVE can read only one PSUM operand; stage O through SBUF)
            osb = psmall.tile([128, TAU], dt.float32, tag="osb")
            nc.scalar.copy(osb[:], ops_[:])
            xc = pprod.tile([128, T], dt.float32, tag="t3")
            nc.vector.tensor_tensor(xc[:, NLAG - 1 : T], eps[:], osb[:], Alu.add)
            nc.vector.tensor_tensor(
                xc[:, NLAG - 2 :: -1], eps[:, 1:TAU], osb[:, 1:TAU], Alu.subtract
            )

            # channel smoothing: ma = W @ xcor  (banded avg-pool matrix, fp32)
            wps = wp.tile([128, T], dt.float32, tag="wp")
            nc.tensor.matmul(
                wps[:, 0:512], wtt[:], xc[:, 0:512], start=True, stop=True
            )
            nc.tensor.matmul(
                wps[:, 512:T], wtt[:], xc[:, 512:T], start=True, stop=True
            )
            ma = pprod.tile([128, T], dt.float32, tag="t2")
            nc.scalar.copy(ma[:], wps[:])

            # peak pick
            nm = pprod.tile([128, T], dt.float32, tag="t1")
            nc.scalar.mul(nm[:], ma[:], -1.0)
            mx8 = psmall.tile([128, 8], dt.float32, tag="mx8")
            mi8 = psmall.tile([128, 8], dt.uint32, tag="mi8")
            nx8 = psmall.tile([128, 8], dt.float32, tag="nx8")
            ni8 = psmall.tile([128, 8], dt.uint32, tag="ni8")
            nc.vector.max(mx8[:], ma[:])
            nc.vector.max_index(mi8[:], mx8[:], ma[:])
            nc.vector.max(nx8[:], nm[:])
            nc.vector.max_index(ni8[:], nx8[:], nm[:])

            vmax = mx8[:, 0:1]
            vnegm = nx8[:, 0:1]  # = -vmin
            sc = psmall.tile([128, 4], dt.float32, tag="sc")
            vmin = sc[:, 2:3]
            idxf = sc[:, 3:4]
            maskt = psmall.tile([128, 1], dt.uint8, tag="msk")
            mask = maskt[:]
            # |vmin| > vmax  ==  (-vmin) > vmax   (since vmax >= vmin)
            nc.vector.tensor_tensor(mask, vnegm, vmax, Alu.is_gt)
            nc.vector.tensor_scalar(vmin, vnegm, -1.0, None, Alu.mult)
            # cc
            nc.vector.tensor_copy(ccs[:, b : b + 1], vmax)
            nc.vector.copy_predicated(ccs[:, b : b + 1], mask, vmin)
            # idx -> tshift
            nc.vector.tensor_copy(idxf, mi8[:, 0:1])
            fmin = sc[:, 0:1]
            nc.vector.tensor_copy(fmin, ni8[:, 0:1])
            nc.vector.copy_predicated(idxf, mask, fmin)
            nc.vector.tensor_scalar(
                tss[:, b : b + 1], idxf, float(NLAG - 1), DT_S, Alu.subtract, Alu.mult
            )

        def _load_bases():
            load_basis(bA, baseA)
            load_basis(bB, baseB)

        pend = None
        for b in range(PB):
            eo_out = emit_head(b, pre_eo_hook=_load_bases if b == 0 else None)
            if pend is not None:
                emit_tail(b - 1, *pend)
            pend = eo_out
        emit_tail(PB - 1, *pend)

        nc.sync.dma_start(cc_o.rearrange("b c -> c b"), ccs[:])
        nc.sync.dma_start(ts_o.rearrange("b c -> c b"), tss[:])

    nc.compile()
    return nc


_CACHE = {}


def _get_compiled():
    if "nc" not in _CACHE:
        _CACHE["nc"] = _build_bass()
        _CACHE["consts"] = _build_consts()
    return _CACHE["nc"], _CACHE["consts"]


def run(data1, data2, trace=False):
    nc, (basisA, basisB, WT, ident) = _get_compiled()
    data1 = np.ascontiguousarray(data1, dtype=np.float32)
    data2 = np.ascontiguousarray(data2, dtype=np.float32)
    in_maps = []
    for core in range(NCORES):
        sl = slice(core * PB, (core + 1) * PB)
        in_maps.append(
            {
                "data1": data1[sl],
                "data2": data2[sl],
                "basisA": basisA,
                "basisB": basisB,
                "wt": WT,
                "ident": ident,
            }
        )
    br = run_bass_kernel_spmd(nc, in_maps, list(range(NCORES)), trace=trace)
    cc = np.concatenate([r["cc"] for r in br.results], axis=0)
    ts = np.concatenate([r["tshift"] for r in br.results], axis=0)
    return cc, ts, br


def kernel(data1, data2, event1, event2):
    cc, ts, _ = run(np.asarray(data1), np.asarray(data2))
    return cc, ts, np.asarray(event1), np.asarray(event2)


# revision 19
# speedup vs baseline: 1.1118x; 1.1118x over previous
"""Trainium2 Bass kernel for nn_CCModel (cross-correlation peak picker).

Math (per (b, c) row):
  X = conj(c1) * c2                       (complex cross-spectrum, F=2049 bins)
  xt = irfft(X, n=4096); roll; slice      -> 599 lags  [-299, 299]
  moving average over channels (window 20, zero-padded)
  peak pick (max vs |min|) -> cc, tshift

Device realization per pair:
  irfft+slice == matmul with cos/sin basis. Using lag symmetry:
    E[c,tau] = sum_k Xr[c,k] * (ck/N) cos(2 pi k tau / N)   tau in [0,300)
    O[c,tau] = sum_k Xi[c,k] * -(ck/N) sin(2 pi k tau / N)
    xcor[299+tau] = E + O ; xcor[299-tau] = E - O
  Xr = r1 r2 + i1 i2  (the add is fused into the PE transpose via PSUM
  accumulation), Xi = r1 i2 - i1 r2.
  Smoothing = banded [128,128] matmul W^T applied after combine.

All matmuls run in plain fp32: float32r (the full-rate PE mode) is only an
11-bit-mantissa format on TRN2, which would perturb the peak picks; fp32 at
4 cycles/row keeps tshift bit-exact vs the jax reference. Emission is
software-pipelined: each pair's combine/smooth/peak tail is deferred one
pair so the per-engine FIFOs never stall the next pair's products.

Sharding: pure data-parallel, 4 of the 32 batch pairs per NeuronCore.
"""

import sys

sys.path.insert(0, "/opt/trn_rl_repo")

import numpy as np
from contextlib import ExitStack

import concourse.bacc as bacc
import concourse.tile as tile
import concourse.mybir as mybir
from concourse.bass_utils import run_bass_kernel_spmd

dt = mybir.dt
Alu = mybir.AluOpType

# hardcoded problem shape
B, C, F = 32, 128, 2049
NCORES = 8
PB = B // NCORES  # pairs per core
NFAST = 4096
NLAG = 300  # int(3.0 / 0.01)
T = 2 * NLAG - 1  # 599
NMA = 20
DT_S = 0.01
TAU = NLAG  # 300 basis columns

# measured f32r rounding emulation for host-side constants (set from probe:
# number of mantissa bits kept; None -> keep full fp32 bits)
F32R_MBITS = None
F32R_RTN = True


def _round_f32r_host(a):
    if F32R_MBITS is None or F32R_MBITS >= 23:
        return a.astype(np.float32)
    bits = np.ascontiguousarray(a, dtype=np.float32).view(np.uint32)
    shift = np.uint32(23 - F32R_MBITS)
    mask = np.uint32(0xFFFFFFFF) << shift
    if F32R_RTN:
        half = np.uint32(1) << np.uint32(22 - F32R_MBITS)
        bits = (bits + half) & mask
    else:
        bits = bits & mask
    return bits.view(np.float32).copy()


def _build_consts():
    k = np.arange(F, dtype=np.int64)
    tau = np.arange(TAU, dtype=np.int64)
    ck = np.where((k == 0) | (k == NFAST // 2), 1.0, 2.0)
    phase = (k[:, None] * tau[None, :]) % NFAST  # exact ints
    ang = (2.0 * np.pi / NFAST) * phase.astype(np.float64)
    basisA = _round_f32r_host((ck[:, None] / NFAST) * np.cos(ang))
    basisB = _round_f32r_host(-(ck[:, None] / NFAST) * np.sin(ang))

    W = np.zeros((C, C), dtype=np.float64)
    pad = NMA // 2
    for c in range(C):
        W[c, max(0, c - pad) : min(C, c + pad)] = 1.0 / NMA
    WT = np.ascontiguousarray(W.T).astype(np.float32)
    ident = np.eye(128, dtype=np.float32)
    return basisA, basisB, WT, ident


def _build_bass():
    nc = bacc.Bacc("TRN2", target_bir_lowering=False, debug=False)
    d1 = nc.dram_tensor("data1", [PB, C, F, 2], dt.float32, kind="ExternalInput").ap()
    d2 = nc.dram_tensor("data2", [PB, C, F, 2], dt.float32, kind="ExternalInput").ap()
    bA = nc.dram_tensor("basisA", [F, TAU], dt.float32, kind="ExternalInput").ap()
    bB = nc.dram_tensor("basisB", [F, TAU], dt.float32, kind="ExternalInput").ap()
    wt = nc.dram_tensor("wt", [C, C], dt.float32, kind="ExternalInput").ap()
    idn = nc.dram_tensor("ident", [128, 128], dt.float32, kind="ExternalInput").ap()
    cc_o = nc.dram_tensor("cc", [PB, C], dt.float32, kind="ExternalOutput").ap()
    ts_o = nc.dram_tensor("tshift", [PB, C], dt.float32, kind="ExternalOutput").ap()

    with tile.TileContext(nc) as tc, ExitStack() as ctx:
        consts = ctx.enter_context(tc.tile_pool(name="consts", bufs=1))
        praw = ctx.enter_context(tc.tile_pool(name="raw", bufs=4))
        pprod = ctx.enter_context(tc.tile_pool(name="prod", bufs=2))
        pzt = ctx.enter_context(tc.tile_pool(name="zt", bufs=1))
        psmall = ctx.enter_context(tc.tile_pool(name="small", bufs=2))
        pout = ctx.enter_context(tc.tile_pool(name="outs", bufs=1))
        tp = ctx.enter_context(tc.tile_pool(name="tp", bufs=3, space="PSUM"))
        eo = ctx.enter_context(tc.tile_pool(name="eo", bufs=4, space="PSUM"))
        wp = ctx.enter_context(tc.tile_pool(name="wp", bufs=1, space="PSUM"))

        # resident constants (basis DMAs are emitted after pair 0's data
        # loads -- they are not needed until the first E matmul)
        baseA = consts.tile([128, 17 * TAU], dt.float32, tag="baseA")
        baseB = consts.tile([128, 17 * TAU], dt.float32, tag="baseB")

        def load_basis(dram, tile_):
            nc.sync.dma_start(
                tile_[:, 0 : 16 * TAU].rearrange("p (j t) -> p j t", j=16),
                dram[0 : 16 * 128].rearrange("(j p) t -> p j t", p=128),
            )
            nc.sync.dma_start(tile_[0:1, 16 * TAU : 17 * TAU], dram[2048:2049])

        wtt = consts.tile([128, 128], dt.float32, tag="wt")
        nc.sync.dma_start(wtt[:], wt)
        ident = consts.tile([128, 128], dt.float32, tag="ident")
        nc.sync.dma_start(ident[:], idn)

        ccs = pout.tile([128, PB], dt.float32, tag="ccs")
        tss = pout.tile([128, PB], dt.float32, tag="tss")

        def emit_head(b, pre_eo_hook=None):
            """DMA + products + transposes + E/O matmuls for pair b."""
            halves = ((0, F),)
            raw1 = praw.tile([128, F, 2], dt.float32, tag="raw")
            raw2 = praw.tile([128, F, 2], dt.float32, tag="raw")
            for lo, hi in halves:
                nc.sync.dma_start(raw1[:, lo:hi, :], d1[b][:, lo:hi, :])
                nc.sync.dma_start(raw2[:, lo:hi, :], d2[b][:, lo:hi, :])

            t1 = pprod.tile([128, F], dt.float32, tag="t1")
            t2 = pprod.tile([128, F], dt.float32, tag="t2")
            t3 = pprod.tile([128, F], dt.float32, tag="t3")
            t4 = pprod.tile([128, F], dt.float32, tag="t4")
            zi = pprod.tile([128, F], dt.float32, tag="t4")
            for lo, hi in halves:
                r1 = raw1[:, lo:hi, 0]
                i1 = raw1[:, lo:hi, 1]
                r2 = raw2[:, lo:hi, 0]
                i2 = raw2[:, lo:hi, 1]
                # cross-spectrum products; t1+t2 add is fused into PE below
                nc.vector.tensor_tensor(t1[:, lo:hi], r1, r2, Alu.mult)
                nc.gpsimd.tensor_tensor(t2[:, lo:hi], i1, i2, Alu.mult)
                nc.vector.tensor_tensor(t3[:, lo:hi], r1, i2, Alu.mult)
                nc.gpsimd.tensor_tensor(t4[:, lo:hi], i1, r2, Alu.mult)
            nc.vector.tensor_tensor(zi[:], t3[:], t4[:], Alu.subtract)  # Xi

            def transposes(srcs, dstt):
                # transpose to k-major, accumulating multiple sources in PSUM
                for g in range(4):
                    pz = tp.tile([128, 512], dt.float32, tag="tp")
                    for q in range(4):
                        j = 4 * g + q
                        for si, src in enumerate(srcs):
                            nc.tensor.matmul(
                                pz[:, 128 * q : 128 * (q + 1)],
                                src[:, 128 * j : 128 * (j + 1)],
                                ident[:],
                                is_transpose=True,
                                start=(si == 0),
                                stop=(si == len(srcs) - 1),
                            )
                    nc.scalar.copy(dstt[:, 512 * g : 512 * (g + 1)], pz[:])
                # the k=2048 leftover row
                pz = tp.tile([128, 512], dt.float32, tag="tp")
                for si, src in enumerate(srcs):
                    nc.tensor.matmul(
                        pz[0:1, 0:128],
                        src[:, 2048:2049],
                        ident[:],
                        is_transpose=True,
                        start=(si == 0),
                        stop=(si == len(srcs) - 1),
                    )
                nc.scalar.copy(dstt[0:1, 16 * 128 : 17 * 128], pz[0:1, 0:128])

            def eo_matmuls(dstp, zt_, base):
                for j in range(16):
                    nc.tensor.matmul(
                        dstp[:],
                        zt_[:, 128 * j : 128 * (j + 1)],
                        base[:, TAU * j : TAU * (j + 1)],
                        start=(j == 0),
                        stop=False,
                    )
                nc.tensor.matmul(
                    dstp[:],
                    zt_[0:1, 16 * 128 : 17 * 128],
                    base[0:1, 16 * TAU : 17 * TAU],
                    start=False,
                    stop=True,
                )

            zrt = pzt.tile([128, 17 * 128], dt.float32, tag="zrt")
            transposes((t1, t2), zrt)
            if pre_eo_hook is not None:
                # pair 0: basis DMAs must be emitted before the first basis
                # read so Tile records the write->read dependency
                pre_eo_hook()
            eps = eo.tile([128, TAU], dt.float32, tag="eo")
            eo_matmuls(eps, zrt, baseA)
            zit = pzt.tile([128, 17 * 128], dt.float32, tag="zit")
            transposes((zi,), zit)
            ops_ = eo.tile([128, TAU], dt.float32, tag="eo")
            eo_matmuls(ops_, zit, baseB)
            return eps, ops_

        def emit_tail(b, eps, ops_):
            """combine + smoothing + peak pick for pair b (deferred a pair)."""
            # combine: xcor[299+tau] = E + O, xcor[299-tau] = E - O
            # (DVE can read only one PSUM operand; stage O through SBUF)
            osb = psmall.tile([128, TAU], dt.float32, tag="osb")
            nc.scalar.copy(osb[:], ops_[:])
            xc = pprod.tile([128, T], dt.float32, tag="t3")
            nc.vector.tensor_tensor(xc[:, NLAG - 1 : T], eps[:], osb[:], Alu.add)
            nc.vector.tensor_tensor(
                xc[:, NLAG - 2 :: -1], eps[:, 1:TAU], osb[:, 1:TAU], Alu.subtract
            )

            # channel smoothing: ma = W @ xcor  (banded avg-pool matrix,
            # fp32). Two passes through ONE psum bank to free a bank for
            # triple-buffered transposes.
            wps = wp.tile([128, 512], dt.float32, tag="wp")
            ma = pprod.tile([128, T], dt.float32, tag="t2")
            nc.tensor.matmul(
                wps[:, 0:512], wtt[:], xc[:, 0:512], start=True, stop=True
            )
            nc.scalar.copy(ma[:, 0:512], wps[:, 0:512])
            nc.tensor.matmul(
                wps[:, 0 : T - 512], wtt[:], xc[:, 512:T], start=True, stop=True
            )
            nc.scalar.copy(ma[:, 512:T], wps[:, 0 : T - 512])

            # peak pick
            nm = pprod.tile([128, T], dt.float32, tag="t1")
            nc.scalar.mul(nm[:], ma[:], -1.0)
            mx8 = psmall.tile([128, 8], dt.float32, tag="mx8")
            mi8 = psmall.tile([128, 8], dt.uint32, tag="mi8")
            nx8 = psmall.tile([128, 8], dt.float32, tag="nx8")
            ni8 = psmall.tile([128, 8], dt.uint32, tag="ni8")
            nc.vector.max(mx8[:], ma[:])
            nc.vector.max_index(mi8[:], mx8[:], ma[:])
            nc.vector.max(nx8[:], nm[:])
            nc.vector.max_index(ni8[:], nx8[:], nm[:])

            vmax = mx8[:, 0:1]
            vnegm = nx8[:, 0:1]  # = -vmin
            sc = psmall.tile([128, 4], dt.float32, tag="sc")
            vmin = sc[:, 2:3]
            idxf = sc[:, 3:4]
            maskt = psmall.tile([128, 1], dt.uint8, tag="msk")
            mask = maskt[:]
            # |vmin| > vmax  ==  (-vmin) > vmax   (since vmax >= vmin)
            nc.vector.tensor_tensor(mask, vnegm, vmax, Alu.is_gt)
            nc.vector.tensor_scalar(vmin, vnegm, -1.0, None, Alu.mult)
            # cc
            nc.vector.tensor_copy(ccs[:, b : b + 1], vmax)
            nc.vector.copy_predicated(ccs[:, b : b + 1], mask, vmin)
            # idx -> tshift
            nc.vector.tensor_copy(idxf, mi8[:, 0:1])
            fmin = sc[:, 0:1]
            nc.vector.tensor_copy(fmin, ni8[:, 0:1])
            nc.vector.copy_predicated(idxf, mask, fmin)
            nc.vector.tensor_scalar(
                tss[:, b : b + 1], idxf, float(NLAG - 1), DT_S, Alu.subtract, Alu.mult
            )

        def _load_bases():
            load_basis(bA, baseA)
            load_basis(bB, baseB)

        pend = None
        for b in range(PB):
            eo_out = emit_head(b, pre_eo_hook=_load_bases if b == 0 else None)
            if pend is not None:
                emit_tail(b - 1, *pend)
            pend = eo_out
        emit_tail(PB - 1, *pend)

        nc.sync.dma_start(cc_o.rearrange("b c -> c b"), ccs[:])
        nc.sync.dma_start(ts_o.rearrange("b c -> c b"), tss[:])

    nc.compile()
    return nc


_CACHE = {}


def _get_compiled():
    if "nc" not in _CACHE:
        _CACHE["nc"] = _build_bass()
        _CACHE["consts"] = _build_consts()
    return _CACHE["nc"], _CACHE["consts"]


def run(data1, data2, trace=False):
    nc, (basisA, basisB, WT, ident) = _get_compiled()
    data1 = np.ascontiguousarray(data1, dtype=np.float32)
    data2 = np.ascontiguousarray(data2, dtype=np.float32)
    in_maps = []
    for core in range(NCORES):
        sl = slice(core * PB, (core + 1) * PB)
        in_maps.append(
            {
                "data1": data1[sl],
                "data2": data2[sl],
                "basisA": basisA,
                "basisB": basisB,
                "wt": WT,
                "ident": ident,
            }
        )
    br = run_bass_kernel_spmd(nc, in_maps, list(range(NCORES)), trace=trace)
    cc = np.concatenate([r["cc"] for r in br.results], axis=0)
    ts = np.concatenate([r["tshift"] for r in br.results], axis=0)
    return cc, ts, br


def kernel(data1, data2, event1, event2):
    cc, ts, _ = run(np.asarray(data1), np.asarray(data2))
    return cc, ts, np.asarray(event1), np.asarray(event2)
